# revision 8
# baseline (speedup 1.0000x reference)
"""BoxMatchKDD Trainium2 kernel.

Pipeline (per core, 8 samples):
  host: sort students/teachers by x1, compute per-tile candidate bands
        (provable superset of all pairs with nonzero x-overlap), arrange
        per-tile teacher data.
  device: for each teacher tile (2 samples x 64 teachers on 128 partitions),
        compute x/y interval overlaps against the banded student window via
        tensor_scalar/scalar_tensor_tensor ops, I = inter area,
        d = log(I) - log(areaA+areaB)  (monotone in IoU: iou = r/(1-r),
        r = I/P), reduce-max d + argmax via MAX_INDEX, gather the matched
        student logits by indirect DMA, softmax/KL in closed form,
        confidence weight w, per-teacher contributions out to DRAM.
  host: final (order-invariant) reduction to the scalar loss.

Out-of-band students provably have inter == 0 -> iou == 0, which can never
pass the keep threshold (0.5); when no candidate passes, keep = 0 and the
argmax choice is multiplied by 0, so banding is exact.
"""

import os

import numpy as np

import concourse.bass as bass
import concourse.bacc as bacc
import concourse.mybir as mybir
from concourse import tile
from concourse.bass import IndirectOffsetOnAxis
from concourse.bass_utils import run_bass_kernel_spmd

F32 = mybir.dt.float32
I32 = mybir.dt.int32
U32 = mybir.dt.uint32
ALU = mybir.AluOpType
ACTF = mybir.ActivationFunctionType

TAU = 2.0
GAMMA = 0.7
EPS = 1e-6
NEG_BIG = -3.0e38  # ~MaxNeg, used as accum init / pad
LOG_THIRD = float(np.log(1.0 / 3.0))  # iou >= 0.5  <=>  I/P >= 1/3
N_CORES = 8
HALF = 64  # teachers per half-tile (one sample)


# ----------------------------------------------------------------- geometry
class Geom:
    pass


def _plan(inputs):
    """Host prep: global tile/band geometry (uniform across cores) and
    per-core device arrays."""
    t_boxes = np.asarray(inputs["t_boxes"], np.float64)
    s_boxes = np.asarray(inputs["s_boxes"], np.float64)
    t_logits = np.asarray(inputs["t_logits"], np.float32)
    s_logits = np.asarray(inputs["s_logits"], np.float32)
    t_valid = np.asarray(inputs["t_valid"], bool)
    s_valid = np.asarray(inputs["s_valid"], bool)

    N, T, _ = t_boxes.shape
    S = s_boxes.shape[1]
    C = t_logits.shape[2]
    spc = N // N_CORES  # samples per core
    pairs = spc // 2
    full_per_pair = T // HALF  # full tiles per pair
    runt = T - full_per_pair * HALF  # leftover teachers per sample
    n_tiles = pairs * full_per_pair + (1 if runt else 0)

    g = Geom()
    g.N, g.T, g.S, g.C = N, T, S, C
    g.spc, g.pairs = spc, pairs
    g.full_per_pair, g.runt, g.n_tiles = full_per_pair, runt, n_tiles

    # --- per-sample sorts -------------------------------------------------
    sb = s_boxes.copy()
    # degenerate far-away box for invalid students: iou == 0 against
    # everything, area 0, sorts to the end (outside every band).
    sb[~s_valid] = 1.0e9
    s_ord = np.argsort(sb[:, :, 0], axis=1, kind="stable")  # by bx1
    tb = t_boxes
    t_ord = np.argsort(tb[:, :, 0], axis=1, kind="stable")  # by ax1

    sbx1 = np.take_along_axis(sb[:, :, 0], s_ord, 1)
    sbx2 = np.take_along_axis(sb[:, :, 2], s_ord, 1)
    sby1 = np.take_along_axis(sb[:, :, 1], s_ord, 1)
    sby2 = np.take_along_axis(sb[:, :, 3], s_ord, 1)
    s_area = np.clip(sbx2 - sbx1, 0, None) * np.clip(sby2 - sby1, 0, None)
    s_area = np.where(
        np.take_along_axis(s_valid, s_ord, 1), s_area, 0.0
    )  # degenerate -> 0 (value irrelevant, excluded)

    tax1 = np.take_along_axis(tb[:, :, 0], t_ord, 1)
    tay1 = np.take_along_axis(tb[:, :, 1], t_ord, 1)
    tax2 = np.take_along_axis(tb[:, :, 2], t_ord, 1)
    tay2 = np.take_along_axis(tb[:, :, 3], t_ord, 1)
    t_area = (tax2 - tax1) * (tay2 - tay1)
    tval_s = np.take_along_axis(t_valid, t_ord, 1).astype(np.float64)

    g.s_ord, g.t_ord = s_ord, t_ord

    # widest valid student box (x), global, + margin
    wbx = np.where(s_valid, s_boxes[:, :, 2] - s_boxes[:, :, 0], 0.0)
    wbx_max = float(wbx.max()) + 1.0

    # --- bands: tile k covers sorted teachers [k0, k1) of every sample ----
    def band(k0, k1):
        lo_px = (tax1[:, k0:k1].min() if k1 > k0 else 0.0) - wbx_max
        hi_px = tax2[:, k0:k1].max() + 1.0
        j_lo = S
        j_hi = 0
        for n in range(N):
            j_lo = min(j_lo, int(np.searchsorted(sbx1[n], lo_px, "left")))
            j_hi = max(j_hi, int(np.searchsorted(sbx1[n], hi_px, "right")))
        j_lo = max(0, j_lo - 1) & ~1
        W = max(8, j_hi - j_lo)
        W += W % 2
        if j_lo + W > S:
            if W > S:
                W, j_lo = S + (S % 2), 0
            else:
                j_lo = S - W
        return j_lo, W

    bands = []
    for k in range(full_per_pair):
        bands.append(band(k * HALF, (k + 1) * HALF))
    bands = bands * pairs  # same band per tile index k for every pair
    # reorder to global tile ids: gid = g*full_per_pair + k
    bands = [bands[k] for _g in range(pairs) for k in range(full_per_pair)]
    if runt:
        bands.append(band(full_per_pair * HALF, T))
    g.bands = bands
    g.Wmax = max(W for _, W in bands)

    # --- tile -> (sample, teacher) map (within a core), rows 0..127 -------
    # full tile gid = g*full_per_pair + k: row p -> sample 2g + p//HALF,
    #   sorted-teacher HALF*k + p%HALF
    # runt tile: row p (<runt*spc) -> sample p//runt, teacher
    #   full_per_pair*HALF + p%runt
    tile_sample = np.zeros((n_tiles, 128), np.int64)  # sample index in core
    tile_teach = np.zeros((n_tiles, 128), np.int64)  # sorted teacher index
    tile_live = np.zeros((n_tiles, 128), bool)
    for gp in range(pairs):
        for k in range(full_per_pair):
            gid = gp * full_per_pair + k
            p = np.arange(128)
            tile_sample[gid] = 2 * gp + p // HALF
            tile_teach[gid] = HALF * k + p % HALF
            tile_live[gid] = True
    if runt:
        gid = n_tiles - 1
        p = np.arange(128)
        live = p < runt * spc
        tile_sample[gid] = np.where(live, p // max(runt, 1), 0)
        tile_teach[gid] = np.where(live, full_per_pair * HALF + p % max(runt, 1), 0)
        tile_live[gid] = live
    g.tile_sample, g.tile_teach, g.tile_live = tile_sample, tile_teach, tile_live

    # --- per-core arrays --------------------------------------------------
    cores = []
    for c in range(N_CORES):
        s0 = c * spc
        ns = slice(s0, s0 + spc)
        # COLS_T [128, 7, n_tiles] partition-major
        cols = np.zeros((128, 7, n_tiles), np.float32)
        for gid in range(n_tiles):
            sm = s0 + tile_sample[gid]
            tt = tile_teach[gid]
            lv = tile_live[gid]
            ax2 = tax2[sm, tt]
            nax1 = -tax1[sm, tt]
            ay2 = tay2[sm, tt]
            nay1 = -tay1[sm, tt]
            aA = t_area[sm, tt]
            base = (tile_sample[gid] * S).astype(np.float64)
            tv = tval_s[sm, tt]
            dead = ~lv
            ax2 = np.where(dead, -1e9, ax2)
            nax1 = np.where(dead, -1e9, nax1)
            ay2 = np.where(dead, -1e9, ay2)
            nay1 = np.where(dead, -1e9, nay1)
            aA = np.where(dead, 1.0, aA)
            base = np.where(dead, 0.0, base)
            tv = np.where(dead, 0.0, tv)
            cols[:, :, gid] = np.stack(
                [ax2, nax1, ay2, nay1, aA, base, tv], axis=0
            ).T.astype(np.float32)

        # ROWS [pairs, 2, 5, S]: bx2, nbx1, by2, nby1, areaB (sorted)
        rows = np.zeros((pairs, 2, 5, S), np.float32)
        for gp in range(pairs):
            for h in (0, 1):
                n = s0 + 2 * gp + h
                rows[gp, h, 0] = sbx2[n]
                rows[gp, h, 1] = -sbx1[n]
                rows[gp, h, 2] = sby2[n]
                rows[gp, h, 3] = -sby1[n]
                rows[gp, h, 4] = s_area[n]

        # TLS [n_tiles, 128, C]: teacher logits in tile layout
        tls = np.zeros((n_tiles, 128, C), np.float32)
        for gid in range(n_tiles):
            sm = s0 + tile_sample[gid]
            tor = t_ord[sm, tile_teach[gid]]
            tls[gid] = t_logits[sm, tor]
            tls[gid][~tile_live[gid]] = 0.0

        # SLS [spc*S, C]: student logits, sorted order per sample
        sls = np.zeros((spc * S, C), np.float32)
        for i, n in enumerate(range(s0, s0 + spc)):
            sls[i * S : (i + 1) * S] = s_logits[n][s_ord[n]]

        cores.append(
            dict(
                COLS=np.ascontiguousarray(cols),
                ROWS=rows,
                TLS=tls,
                SLS=sls,
            )
        )
    g.cores = cores
    return g


# ----------------------------------------------------------------- program
def _build(g):
    nc = bacc.Bacc()
    S, C, nt = g.S, g.C, g.n_tiles
    Wmax = g.Wmax

    COLS = nc.dram_tensor("COLS", [128, 7, nt], F32, kind="ExternalInput")
    ROWS = nc.dram_tensor("ROWS", [g.pairs, 2, 5, S], F32, kind="ExternalInput")
    TLS = nc.dram_tensor("TLS", [nt, 128, C], F32, kind="ExternalInput")
    SLS = nc.dram_tensor("SLS", [g.spc * S, C], F32, kind="ExternalInput")
    OUT = nc.dram_tensor("OUT", [4, 128, nt], F32, kind="ExternalOutput")

    def rows_bcast_ap(sample0, nsamp, q, rep):
        # DRAM AP reading ROWS[sample//2, sample%2, q, :] for `nsamp`
        # consecutive samples, each replicated `rep` times along partitions
        # (0-stride). One DMA -> one completion semaphore.
        off = (sample0 * 5 + q) * S
        return bass.AP(ROWS, off, [[5 * S, nsamp], [0, rep], [1, S]])

    with tile.TileContext(nc) as tc:
        with (
            tc.tile_pool(name="bc", bufs=2) as bcp,
            tc.tile_pool(name="mat", bufs=2) as mp,
            tc.tile_pool(name="cols", bufs=1) as cp,
            tc.tile_pool(name="kl", bufs=3) as kp,
        ):
            # --- persistent column bank + accumulators ---
            colbank = cp.tile([128, 7 * nt], F32, tag="colbank")
            nc.sync.dma_start(out=colbank[:], in_=COLS[:, :, :])

            def col(q):
                return colbank[:, q * nt : (q + 1) * nt]

            def colv(q, gid):
                return colbank[:, q * nt + gid : q * nt + gid + 1]

            join = cp.tile([128, 4], F32, tag="join")
            nc.vector.tensor_copy(out=join[:, 0:1], in_=colbank[:, 0:1])
            nc.scalar.copy(out=join[:, 1:2], in_=colbank[:, 0:1])

            mbuf = cp.tile([128, nt], F32, tag="mbuf")
            max8 = cp.tile([128, 8 * nt], F32, tag="max8")
            jbuf = cp.tile([128, 8 * nt], U32, tag="jbuf")
            stb = cp.tile([128, nt], F32, tag="stb")
            ssb = cp.tile([128, nt], F32, tag="ssb")
            a1b = cp.tile([128, nt], F32, tag="a1b")
            a2b = cp.tile([128, nt], F32, tag="a2b")
            tmx = cp.tile([128, nt], F32, tag="tmx")


            # --- matrix stage ---
            def process(gid, bc):
                lo, W = g.bands[gid]
                u = mp.tile([128, Wmax], F32, tag="u")
                v = mp.tile([128, Wmax], F32, tag="v")
                wx0 = mp.tile([128, Wmax], F32, tag="wx0")
                wy0 = mp.tile([128, Wmax], F32, tag="wy0")
                ii = mp.tile([128, Wmax], F32, tag="ii")
                li = mp.tile([128, Wmax], F32, tag="li")
                lp = mp.tile([128, Wmax], F32, tag="lp")
                dd = mp.tile([128, Wmax], F32, tag="dd")
                win = slice(lo, lo + W)
                nc.vector.tensor_scalar(
                    out=u[:, :W], in0=bc[0][:, win], scalar1=colv(0, gid),
                    scalar2=None, op0=ALU.min,
                )
                nc.vector.scalar_tensor_tensor(
                    out=wx0[:, :W], in0=bc[1][:, win], scalar=colv(1, gid),
                    in1=u[:, :W], op0=ALU.min, op1=ALU.add,
                )
                nc.vector.tensor_scalar(
                    out=v[:, :W], in0=bc[2][:, win], scalar1=colv(2, gid),
                    scalar2=None, op0=ALU.min,
                )
                nc.vector.scalar_tensor_tensor(
                    out=wy0[:, :W], in0=bc[3][:, win], scalar=colv(3, gid),
                    in1=v[:, :W], op0=ALU.min, op1=ALU.add,
                )
                # I = max(wx0,0)*wy0  (<=0 or NaN only when true inter==0;
                # log() of those is -inf/NaN and MAX suppresses NaN)
                nc.vector.scalar_tensor_tensor(
                    out=ii[:, :W], in0=wx0[:, :W], scalar=0.0,
                    in1=wy0[:, :W], op0=ALU.max, op1=ALU.mult,
                )
                nc.scalar.activation(out=li[:, :W], in_=ii[:, :W], func=ACTF.Ln)
                nc.scalar.activation(
                    out=lp[:, :W], in_=bc[4][:, win], func=ACTF.Ln,
                    bias=colv(4, gid), scale=1.0,
                )
                nc.vector.tensor_tensor(
                    out=dd[:, :W], in0=li[:, :W], in1=lp[:, :W],
                    op=ALU.subtract,
                )
                nc.vector.max(
                    out=max8[:, 8 * gid : 8 * gid + 8], in_=dd[:, :W]
                )
                nc.vector.max_index(
                    out=jbuf[:, 8 * gid : 8 * gid + 8],
                    in_max=max8[:, 8 * gid : 8 * gid + 8],
                    in_values=dd[:, :W],
                )

            for gp in range(g.pairs):
                bc = [bcp.tile([128, S], F32, tag=f"bc{q}", name=f"bc{q}") for q in range(5)]
                for q in range(5):
                    nc.sync.dma_start(
                        out=bc[q][:, :], in_=rows_bcast_ap(2 * gp, 2, q, HALF)
                    )
                for k in range(g.full_per_pair):
                    process(gp * g.full_per_pair + k, bc)

            if g.runt:
                bc = [bcp.tile([128, S], F32, tag=f"bc{q}", name=f"bc{q}") for q in range(5)]
                fills = [-1e9, -1e9, -1e9, -1e9, 0.0]
                nrows = g.runt
                for q in range(5):
                    nc.vector.memset(bc[q][:], fills[q])
                    nc.sync.dma_start(
                        out=bc[q][0 : nrows * g.spc, :],
                        in_=rows_bcast_ap(0, g.spc, q, nrows),
                    )
                process(nt - 1, bc)

            # --- batched index/keep math on [128, nt] ---
            jf = cp.tile([128, nt], F32, tag="jf")
            sidx = cp.tile([128, nt], I32, tag="sidx")
            jview = jbuf[:].rearrange("p (t e) -> p t e", e=8)[:, :, 0:1]
            nc.vector.tensor_copy(out=jf[:], in_=jview)
            nc.vector.tensor_scalar(
                out=jf[:], in0=jf[:], scalar1=float(S - 1), scalar2=0.0,
                op0=ALU.min, op1=ALU.max,
            )
            nc.vector.tensor_tensor(
                out=jf[:], in0=jf[:], in1=col(5), op=ALU.add
            )
            nc.vector.tensor_copy(out=sidx[:], in_=jf[:])

            keep = cp.tile([128, nt], F32, tag="keep")
            mview = max8[:].rearrange("p (t e) -> p t e", e=8)[:, :, 0:1]
            nc.vector.tensor_copy(out=mbuf[:], in_=mview)
            nc.vector.tensor_scalar(
                out=keep[:], in0=mbuf[:], scalar1=float(LOG_THIRD),
                scalar2=None, op0=ALU.is_ge,
            )
            nc.vector.tensor_tensor(
                out=keep[:], in0=keep[:], in1=col(6), op=ALU.mult
            )

            # --- KL stage ---
            for gid in range(nt):
                tl = kp.tile([128, C], F32, tag="tl")
                sl = kp.tile([128, C], F32, tag="sl")
                et = kp.tile([128, C], F32, tag="et")
                es = kp.tile([128, C], F32, tag="es")
                dead = kp.tile([128, C], F32, tag="dead")
                nc.sync.dma_start(out=tl[:], in_=TLS[gid, :, :])
                if os.environ.get("BM_NO_GATHER"):
                    nc.sync.dma_start(out=sl[:], in_=SLS[0:128, :])
                else:
                    nc.gpsimd.indirect_dma_start(
                        out=sl[:],
                        out_offset=None,
                        in_=SLS[:],
                        in_offset=IndirectOffsetOnAxis(
                            ap=sidx[:, gid : gid + 1], axis=0
                        ),
                    )
                nc.scalar.activation(
                    out=et[:], in_=tl[:], func=ACTF.Exp, scale=1.0 / TAU,
                    accum_out=stb[:, gid : gid + 1],
                )
                nc.scalar.activation(
                    out=es[:], in_=sl[:], func=ACTF.Exp, scale=1.0 / TAU,
                    accum_out=ssb[:, gid : gid + 1],
                )
                nc.vector.tensor_reduce(
                    out=tmx[:, gid : gid + 1], in_=tl[:],
                    axis=mybir.AxisListType.X, op=ALU.max,
                )
                nc.vector.tensor_copy(out=join[:, 2:3], in_=sl[:, 0:1])
                nc.vector.tensor_tensor(
                    out=dead[:], in0=et[:], in1=tl[:], op=ALU.mult
                )
                nc.vector.tensor_reduce(
                    out=a1b[:, gid : gid + 1], in_=dead[:],
                    axis=mybir.AxisListType.X, op=ALU.add,
                )
                nc.vector.tensor_tensor(
                    out=dead[:], in0=et[:], in1=sl[:], op=ALU.mult
                )
                nc.vector.tensor_reduce(
                    out=a2b[:, gid : gid + 1], in_=dead[:],
                    axis=mybir.AxisListType.X, op=ALU.add,
                )

            # --- batched tail: kl, w, per on [128, nt] ---
            rst = cp.tile([128, nt], F32, tag="rst")
            lst = cp.tile([128, nt], F32, tag="lst")
            lss = cp.tile([128, nt], F32, tag="lss")
            kl = cp.tile([128, nt], F32, tag="kl")
            cb = cp.tile([128, nt], F32, tag="cb")
            w = cp.tile([128, nt], F32, tag="w")
            pk = cp.tile([128, nt], F32, tag="pk")
            nc.vector.reciprocal(out=rst[:], in_=stb[:])
            nc.scalar.activation(out=lst[:], in_=stb[:], func=ACTF.Ln)
            nc.scalar.activation(out=lss[:], in_=ssb[:], func=ACTF.Ln)
            nc.vector.tensor_tensor(out=kl[:], in0=a1b[:], in1=a2b[:], op=ALU.subtract)
            nc.vector.tensor_scalar(
                out=kl[:], in0=kl[:], scalar1=1.0 / TAU, scalar2=None, op0=ALU.mult
            )
            nc.vector.tensor_tensor(out=kl[:], in0=kl[:], in1=rst[:], op=ALU.mult)
            nc.vector.tensor_tensor(out=kl[:], in0=kl[:], in1=lst[:], op=ALU.subtract)
            nc.vector.tensor_tensor(out=kl[:], in0=kl[:], in1=lss[:], op=ALU.add)
            # c = exp(tmax/TAU) / St
            nc.scalar.activation(out=cb[:], in_=tmx[:], func=ACTF.Exp, scale=1.0 / TAU)
            nc.vector.tensor_tensor(out=cb[:], in0=cb[:], in1=rst[:], op=ALU.mult)
            nc.vector.tensor_scalar(
                out=w[:], in0=cb[:], scalar1=float(-GAMMA),
                scalar2=float(1.0 / max(EPS, 1.0 - GAMMA)), op0=ALU.add, op1=ALU.mult,
            )
            nc.vector.tensor_scalar(
                out=w[:], in0=w[:], scalar1=0.0, scalar2=1.0, op0=ALU.max, op1=ALU.min
            )
            nc.vector.tensor_tensor(out=pk[:], in0=w[:], in1=kl[:], op=ALU.mult)
            nc.vector.tensor_scalar(
                out=pk[:], in0=pk[:], scalar1=float(TAU * TAU), scalar2=None,
                op0=ALU.mult,
            )
            nc.vector.tensor_tensor(out=pk[:], in0=pk[:], in1=keep[:], op=ALU.mult)

            nc.sync.dma_start(out=OUT[0, :, :], in_=pk[:])
            nc.sync.dma_start(out=OUT[1, :, :], in_=keep[:])
            nc.sync.dma_start(out=OUT[2, :, :], in_=mbuf[:])
            nc.sync.dma_start(out=OUT[3, :, :], in_=jf[:])
    if not nc.is_finalized():
        nc.finalize()
    return nc


# ----------------------------------------------------------------- combine
def _combine(g, outs):
    """outs: list of per-core OUT arrays [4, 128, nt] -> scalar loss."""
    loss_i = np.zeros(g.N, np.float64)
    cnt = np.zeros(g.N, np.float64)
    for c, o in enumerate(outs):
        pk, keep = np.asarray(o[0], np.float64), np.asarray(o[1], np.float64)
        for gid in range(g.n_tiles):
            lv = g.tile_live[gid]
            sm = c * g.spc + g.tile_sample[gid]
            np.add.at(loss_i, sm[lv], pk[lv, gid])
            np.add.at(cnt, sm[lv], keep[lv, gid])
    safe = np.maximum(cnt, 1.0)
    loss_i = loss_i / safe
    contrib = cnt > 0
    denom = contrib.sum()
    if denom > 0:
        return np.float32(loss_i[contrib].sum() / denom)
    return np.float32(0.0)


# ------------------------------------------------------------------- entry
_CACHE = {}


def kernel(**inputs):
    g = _plan(inputs)
    key = (g.N, g.T, g.S, g.C, tuple(g.bands),
           os.environ.get("BM_NO_GATHER"), os.environ.get("BM_NO_MAXIDX"))
    if key not in _CACHE:
        _CACHE[key] = _build(g)
    nc = _CACHE[key]
    in_maps = [
        {k: np.ascontiguousarray(v) for k, v in g.cores[c].items()}
        for c in range(N_CORES)
    ]
    res = run_bass_kernel_spmd(nc, in_maps, list(range(N_CORES)))
    outs = [res.results[c]["OUT"] for c in range(N_CORES)]
    return _combine(g, outs)


if __name__ == "__main__":
    import reference as R

    inputs = {k: np.asarray(v) for k, v in R.setup_inputs().items()}
    print("loss =", kernel(**inputs))


# revision 9
# speedup vs baseline: 1.0085x; 1.0085x over previous
"""BoxMatchKDD Trainium2 kernel.

Pipeline (per core, 8 samples):
  host: sort students/teachers by x1, compute per-tile candidate bands
        (provable superset of all pairs with nonzero x-overlap), arrange
        per-tile teacher data.
  device: for each teacher tile (2 samples x 64 teachers on 128 partitions),
        compute x/y interval overlaps against the banded student window via
        tensor_scalar/scalar_tensor_tensor ops, I = inter area,
        d = log(I) - log(areaA+areaB)  (monotone in IoU: iou = r/(1-r),
        r = I/P), reduce-max d + argmax via MAX_INDEX, gather the matched
        student logits by indirect DMA, softmax/KL in closed form,
        confidence weight w, per-teacher contributions out to DRAM.
  host: final (order-invariant) reduction to the scalar loss.

Out-of-band students provably have inter == 0 -> iou == 0, which can never
pass the keep threshold (0.5); when no candidate passes, keep = 0 and the
argmax choice is multiplied by 0, so banding is exact.
"""

import os

import numpy as np

import concourse.bass as bass
import concourse.bacc as bacc
import concourse.mybir as mybir
from concourse import tile
from concourse.bass import IndirectOffsetOnAxis
from concourse.bass_utils import run_bass_kernel_spmd

F32 = mybir.dt.float32
I32 = mybir.dt.int32
U32 = mybir.dt.uint32
ALU = mybir.AluOpType
ACTF = mybir.ActivationFunctionType

TAU = 2.0
GAMMA = 0.7
EPS = 1e-6
NEG_BIG = -3.0e38  # ~MaxNeg, used as accum init / pad
LOG_THIRD = float(np.log(1.0 / 3.0))  # iou >= 0.5  <=>  I/P >= 1/3
N_CORES = 8
HALF = 64  # teachers per half-tile (one sample)


# ----------------------------------------------------------------- geometry
class Geom:
    pass


def _plan(inputs):
    """Host prep: global tile/band geometry (uniform across cores) and
    per-core device arrays."""
    t_boxes = np.asarray(inputs["t_boxes"], np.float64)
    s_boxes = np.asarray(inputs["s_boxes"], np.float64)
    t_logits = np.asarray(inputs["t_logits"], np.float32)
    s_logits = np.asarray(inputs["s_logits"], np.float32)
    t_valid = np.asarray(inputs["t_valid"], bool)
    s_valid = np.asarray(inputs["s_valid"], bool)

    N, T, _ = t_boxes.shape
    S = s_boxes.shape[1]
    C = t_logits.shape[2]
    spc = N // N_CORES  # samples per core
    pairs = spc // 2
    full_per_pair = T // HALF  # full tiles per pair
    runt = T - full_per_pair * HALF  # leftover teachers per sample
    n_tiles = pairs * full_per_pair + (1 if runt else 0)

    g = Geom()
    g.N, g.T, g.S, g.C = N, T, S, C
    g.spc, g.pairs = spc, pairs
    g.full_per_pair, g.runt, g.n_tiles = full_per_pair, runt, n_tiles

    # --- per-sample sorts -------------------------------------------------
    sb = s_boxes.copy()
    # degenerate far-away box for invalid students: iou == 0 against
    # everything, area 0, sorts to the end (outside every band).
    sb[~s_valid] = 1.0e9
    s_ord = np.argsort(sb[:, :, 0], axis=1, kind="stable")  # by bx1
    tb = t_boxes
    t_ord = np.argsort(tb[:, :, 0], axis=1, kind="stable")  # by ax1

    sbx1 = np.take_along_axis(sb[:, :, 0], s_ord, 1)
    sbx2 = np.take_along_axis(sb[:, :, 2], s_ord, 1)
    sby1 = np.take_along_axis(sb[:, :, 1], s_ord, 1)
    sby2 = np.take_along_axis(sb[:, :, 3], s_ord, 1)
    s_area = np.clip(sbx2 - sbx1, 0, None) * np.clip(sby2 - sby1, 0, None)
    s_area = np.where(
        np.take_along_axis(s_valid, s_ord, 1), s_area, 0.0
    )  # degenerate -> 0 (value irrelevant, excluded)

    tax1 = np.take_along_axis(tb[:, :, 0], t_ord, 1)
    tay1 = np.take_along_axis(tb[:, :, 1], t_ord, 1)
    tax2 = np.take_along_axis(tb[:, :, 2], t_ord, 1)
    tay2 = np.take_along_axis(tb[:, :, 3], t_ord, 1)
    t_area = (tax2 - tax1) * (tay2 - tay1)
    tval_s = np.take_along_axis(t_valid, t_ord, 1).astype(np.float64)

    g.s_ord, g.t_ord = s_ord, t_ord

    # widest valid student box (x), global, + margin
    wbx = np.where(s_valid, s_boxes[:, :, 2] - s_boxes[:, :, 0], 0.0)
    wbx_max = float(wbx.max()) + 1.0

    # --- bands: tile k covers sorted teachers [k0, k1) of every sample ----
    def band(k0, k1):
        lo_px = (tax1[:, k0:k1].min() if k1 > k0 else 0.0) - wbx_max
        hi_px = tax2[:, k0:k1].max() + 1.0
        j_lo = S
        j_hi = 0
        for n in range(N):
            j_lo = min(j_lo, int(np.searchsorted(sbx1[n], lo_px, "left")))
            j_hi = max(j_hi, int(np.searchsorted(sbx1[n], hi_px, "right")))
        j_lo = max(0, j_lo - 1) & ~1
        W = max(8, j_hi - j_lo)
        W += W % 2
        if j_lo + W > S:
            if W > S:
                W, j_lo = S + (S % 2), 0
            else:
                j_lo = S - W
        return j_lo, W

    bands = []
    for k in range(full_per_pair):
        bands.append(band(k * HALF, (k + 1) * HALF))
    bands = bands * pairs  # same band per tile index k for every pair
    # reorder to global tile ids: gid = g*full_per_pair + k
    bands = [bands[k] for _g in range(pairs) for k in range(full_per_pair)]
    if runt:
        bands.append(band(full_per_pair * HALF, T))
    g.bands = bands
    g.Wmax = max(W for _, W in bands)

    # --- tile -> (sample, teacher) map (within a core), rows 0..127 -------
    # full tile gid = g*full_per_pair + k: row p -> sample 2g + p//HALF,
    #   sorted-teacher HALF*k + p%HALF
    # runt tile: row p (<runt*spc) -> sample p//runt, teacher
    #   full_per_pair*HALF + p%runt
    tile_sample = np.zeros((n_tiles, 128), np.int64)  # sample index in core
    tile_teach = np.zeros((n_tiles, 128), np.int64)  # sorted teacher index
    tile_live = np.zeros((n_tiles, 128), bool)
    for gp in range(pairs):
        for k in range(full_per_pair):
            gid = gp * full_per_pair + k
            p = np.arange(128)
            tile_sample[gid] = 2 * gp + p // HALF
            tile_teach[gid] = HALF * k + p % HALF
            tile_live[gid] = True
    if runt:
        gid = n_tiles - 1
        p = np.arange(128)
        live = p < runt * spc
        tile_sample[gid] = np.where(live, p // max(runt, 1), 0)
        tile_teach[gid] = np.where(live, full_per_pair * HALF + p % max(runt, 1), 0)
        tile_live[gid] = live
    g.tile_sample, g.tile_teach, g.tile_live = tile_sample, tile_teach, tile_live

    # --- per-core arrays --------------------------------------------------
    cores = []
    for c in range(N_CORES):
        s0 = c * spc
        ns = slice(s0, s0 + spc)
        # COLS_T [128, 7, n_tiles] partition-major
        cols = np.zeros((128, 7, n_tiles), np.float32)
        for gid in range(n_tiles):
            sm = s0 + tile_sample[gid]
            tt = tile_teach[gid]
            lv = tile_live[gid]
            ax2 = tax2[sm, tt]
            nax1 = -tax1[sm, tt]
            ay2 = tay2[sm, tt]
            nay1 = -tay1[sm, tt]
            aA = t_area[sm, tt]
            base = (tile_sample[gid] * S).astype(np.float64)
            tv = tval_s[sm, tt]
            dead = ~lv
            ax2 = np.where(dead, -1e9, ax2)
            nax1 = np.where(dead, -1e9, nax1)
            ay2 = np.where(dead, -1e9, ay2)
            nay1 = np.where(dead, -1e9, nay1)
            aA = np.where(dead, 1.0, aA)
            base = np.where(dead, 0.0, base)
            tv = np.where(dead, 0.0, tv)
            cols[:, :, gid] = np.stack(
                [ax2, nax1, ay2, nay1, aA, base, tv], axis=0
            ).T.astype(np.float32)

        # ROWS [pairs, 2, 5, S]: bx2, nbx1, by2, nby1, areaB (sorted)
        rows = np.zeros((pairs, 2, 5, S), np.float32)
        for gp in range(pairs):
            for h in (0, 1):
                n = s0 + 2 * gp + h
                rows[gp, h, 0] = sbx2[n]
                rows[gp, h, 1] = -sbx1[n]
                rows[gp, h, 2] = sby2[n]
                rows[gp, h, 3] = -sby1[n]
                rows[gp, h, 4] = s_area[n]

        # TLS [n_tiles, 128, C]: teacher logits in tile layout
        tls = np.zeros((n_tiles, 128, C), np.float32)
        for gid in range(n_tiles):
            sm = s0 + tile_sample[gid]
            tor = t_ord[sm, tile_teach[gid]]
            tls[gid] = t_logits[sm, tor]
            tls[gid][~tile_live[gid]] = 0.0

        # SLS [spc*S, C]: student logits, sorted order per sample
        sls = np.zeros((spc * S, C), np.float32)
        for i, n in enumerate(range(s0, s0 + spc)):
            sls[i * S : (i + 1) * S] = s_logits[n][s_ord[n]]

        cores.append(
            dict(
                COLS=np.ascontiguousarray(cols),
                ROWS=rows,
                TLS=tls,
                SLS=sls,
            )
        )
    g.cores = cores
    return g


# ----------------------------------------------------------------- program
def _build(g):
    nc = bacc.Bacc()
    S, C, nt = g.S, g.C, g.n_tiles
    Wmax = g.Wmax

    COLS = nc.dram_tensor("COLS", [128, 7, nt], F32, kind="ExternalInput")
    ROWS = nc.dram_tensor("ROWS", [g.pairs, 2, 5, S], F32, kind="ExternalInput")
    TLS = nc.dram_tensor("TLS", [nt, 128, C], F32, kind="ExternalInput")
    SLS = nc.dram_tensor("SLS", [g.spc * S, C], F32, kind="ExternalInput")
    OUT = nc.dram_tensor("OUT", [4, 128, nt], F32, kind="ExternalOutput")

    def rows_bcast_ap(sample0, nsamp, q, rep):
        # DRAM AP reading ROWS[sample//2, sample%2, q, :] for `nsamp`
        # consecutive samples, each replicated `rep` times along partitions
        # (0-stride). One DMA -> one completion semaphore.
        off = (sample0 * 5 + q) * S
        return bass.AP(ROWS, off, [[5 * S, nsamp], [0, rep], [1, S]])

    with tile.TileContext(nc) as tc:
        with (
            tc.tile_pool(name="bc", bufs=2) as bcp,
            tc.tile_pool(name="mat", bufs=2) as mp,
            tc.tile_pool(name="cols", bufs=1) as cp,
            tc.tile_pool(name="kl", bufs=3) as kp,
        ):
            # --- persistent column bank + accumulators ---
            colbank = cp.tile([128, 7 * nt], F32, tag="colbank")
            nc.sync.dma_start(out=colbank[:], in_=COLS[:, :, :])

            def col(q):
                return colbank[:, q * nt : (q + 1) * nt]

            def colv(q, gid):
                return colbank[:, q * nt + gid : q * nt + gid + 1]

            join = cp.tile([128, 4], F32, tag="join")
            nc.vector.tensor_copy(out=join[:, 0:1], in_=colbank[:, 0:1])
            nc.scalar.copy(out=join[:, 1:2], in_=colbank[:, 0:1])

            mbuf = cp.tile([128, nt], F32, tag="mbuf")
            max8 = cp.tile([128, 8 * nt], F32, tag="max8")
            jbuf = cp.tile([128, 8 * nt], U32, tag="jbuf")
            stb = cp.tile([128, nt], F32, tag="stb")
            ssb = cp.tile([128, nt], F32, tag="ssb")
            a1b = cp.tile([128, nt], F32, tag="a1b")
            a2b = cp.tile([128, nt], F32, tag="a2b")
            tmx = cp.tile([128, nt], F32, tag="tmx")


            # --- matrix stage ---
            def process(gid, bc):
                lo, W = g.bands[gid]
                u = mp.tile([128, Wmax], F32, tag="u")
                v = mp.tile([128, Wmax], F32, tag="v")
                wx0 = mp.tile([128, Wmax], F32, tag="wx0")
                wy0 = mp.tile([128, Wmax], F32, tag="wy0")
                ii = mp.tile([128, Wmax], F32, tag="ii")
                li = mp.tile([128, Wmax], F32, tag="li")
                lp = mp.tile([128, Wmax], F32, tag="lp")
                dd = mp.tile([128, Wmax], F32, tag="dd")
                win = slice(lo, lo + W)
                nc.vector.tensor_scalar(
                    out=u[:, :W], in0=bc[0][:, win], scalar1=colv(0, gid),
                    scalar2=None, op0=ALU.min,
                )
                nc.vector.scalar_tensor_tensor(
                    out=wx0[:, :W], in0=bc[1][:, win], scalar=colv(1, gid),
                    in1=u[:, :W], op0=ALU.min, op1=ALU.add,
                )
                nc.vector.tensor_scalar(
                    out=v[:, :W], in0=bc[2][:, win], scalar1=colv(2, gid),
                    scalar2=None, op0=ALU.min,
                )
                nc.vector.scalar_tensor_tensor(
                    out=wy0[:, :W], in0=bc[3][:, win], scalar=colv(3, gid),
                    in1=v[:, :W], op0=ALU.min, op1=ALU.add,
                )
                # I = max(wx0,0)*wy0  (<=0 or NaN only when true inter==0;
                # log() of those is -inf/NaN and MAX suppresses NaN)
                nc.vector.scalar_tensor_tensor(
                    out=ii[:, :W], in0=wx0[:, :W], scalar=0.0,
                    in1=wy0[:, :W], op0=ALU.max, op1=ALU.mult,
                )
                nc.scalar.activation(out=li[:, :W], in_=ii[:, :W], func=ACTF.Ln)
                nc.scalar.activation(
                    out=lp[:, :W], in_=bc[4][:, win], func=ACTF.Ln,
                    bias=colv(4, gid), scale=1.0,
                )
                nc.vector.tensor_tensor(
                    out=dd[:, :W], in0=li[:, :W], in1=lp[:, :W],
                    op=ALU.subtract,
                )
                nc.vector.max(
                    out=max8[:, 8 * gid : 8 * gid + 8], in_=dd[:, :W]
                )
                nc.vector.max_index(
                    out=jbuf[:, 8 * gid : 8 * gid + 8],
                    in_max=max8[:, 8 * gid : 8 * gid + 8],
                    in_values=dd[:, :W],
                )

            for gp in range(g.pairs):
                bc = [bcp.tile([128, S], F32, tag=f"bc{q}", name=f"bc{q}") for q in range(5)]
                for q in range(5):
                    nc.sync.dma_start(
                        out=bc[q][:, :], in_=rows_bcast_ap(2 * gp, 2, q, HALF)
                    )
                for k in range(g.full_per_pair):
                    process(gp * g.full_per_pair + k, bc)

            if g.runt:
                bc = [bcp.tile([128, S], F32, tag=f"bc{q}", name=f"bc{q}") for q in range(5)]
                fills = [-1e9, -1e9, -1e9, -1e9, 0.0]
                nrows = g.runt
                for q in range(5):
                    nc.vector.memset(bc[q][:], fills[q])
                    nc.sync.dma_start(
                        out=bc[q][0 : nrows * g.spc, :],
                        in_=rows_bcast_ap(0, g.spc, q, nrows),
                    )
                process(nt - 1, bc)

            # --- batched index/keep math on [128, nt] ---
            jf = cp.tile([128, nt], F32, tag="jf")
            sidx = cp.tile([128, nt], I32, tag="sidx")
            _jb = jbuf[:]
            jview = bass.AP(_jb.tensor, _jb.offset, [_jb.ap[0], [8, nt]])
            nc.vector.tensor_copy(out=jf[:], in_=jview)
            nc.vector.tensor_scalar(
                out=jf[:], in0=jf[:], scalar1=float(S - 1), scalar2=0.0,
                op0=ALU.min, op1=ALU.max,
            )
            nc.vector.tensor_tensor(
                out=jf[:], in0=jf[:], in1=col(5), op=ALU.add
            )
            nc.vector.tensor_copy(out=sidx[:], in_=jf[:])

            keep = cp.tile([128, nt], F32, tag="keep")
            _m8 = max8[:]
            mview = bass.AP(_m8.tensor, _m8.offset, [_m8.ap[0], [8, nt]])
            nc.vector.tensor_copy(out=mbuf[:], in_=mview)
            nc.vector.tensor_scalar(
                out=keep[:], in0=mbuf[:], scalar1=float(LOG_THIRD),
                scalar2=None, op0=ALU.is_ge,
            )
            nc.vector.tensor_tensor(
                out=keep[:], in0=keep[:], in1=col(6), op=ALU.mult
            )

            # --- KL stage ---
            for gid in range(nt):
                tl = kp.tile([128, C], F32, tag="tl")
                sl = kp.tile([128, C], F32, tag="sl")
                et = kp.tile([128, C], F32, tag="et")
                es = kp.tile([128, C], F32, tag="es")
                dead = kp.tile([128, C], F32, tag="dead")
                nc.sync.dma_start(out=tl[:], in_=TLS[gid, :, :])
                if os.environ.get("BM_NO_GATHER"):
                    nc.sync.dma_start(out=sl[:], in_=SLS[0:128, :])
                else:
                    nc.gpsimd.indirect_dma_start(
                        out=sl[:],
                        out_offset=None,
                        in_=SLS[:],
                        in_offset=IndirectOffsetOnAxis(
                            ap=sidx[:, gid : gid + 1], axis=0
                        ),
                    )
                nc.scalar.activation(
                    out=et[:], in_=tl[:], func=ACTF.Exp, scale=1.0 / TAU,
                    accum_out=stb[:, gid : gid + 1],
                )
                nc.scalar.activation(
                    out=es[:], in_=sl[:], func=ACTF.Exp, scale=1.0 / TAU,
                    accum_out=ssb[:, gid : gid + 1],
                )
                nc.vector.tensor_reduce(
                    out=tmx[:, gid : gid + 1], in_=tl[:],
                    axis=mybir.AxisListType.X, op=ALU.max,
                )
                nc.vector.tensor_copy(out=join[:, 2:3], in_=sl[:, 0:1])
                nc.vector.tensor_tensor(
                    out=dead[:], in0=et[:], in1=tl[:], op=ALU.mult
                )
                nc.vector.tensor_reduce(
                    out=a1b[:, gid : gid + 1], in_=dead[:],
                    axis=mybir.AxisListType.X, op=ALU.add,
                )
                nc.vector.tensor_tensor(
                    out=dead[:], in0=et[:], in1=sl[:], op=ALU.mult
                )
                nc.vector.tensor_reduce(
                    out=a2b[:, gid : gid + 1], in_=dead[:],
                    axis=mybir.AxisListType.X, op=ALU.add,
                )

            # --- batched tail: kl, w, per on [128, nt] ---
            rst = cp.tile([128, nt], F32, tag="rst")
            lst = cp.tile([128, nt], F32, tag="lst")
            lss = cp.tile([128, nt], F32, tag="lss")
            kl = cp.tile([128, nt], F32, tag="kl")
            cb = cp.tile([128, nt], F32, tag="cb")
            w = cp.tile([128, nt], F32, tag="w")
            pk = cp.tile([128, nt], F32, tag="pk")
            nc.vector.reciprocal(out=rst[:], in_=stb[:])
            nc.scalar.activation(out=lst[:], in_=stb[:], func=ACTF.Ln)
            nc.scalar.activation(out=lss[:], in_=ssb[:], func=ACTF.Ln)
            nc.vector.tensor_tensor(out=kl[:], in0=a1b[:], in1=a2b[:], op=ALU.subtract)
            nc.vector.tensor_scalar(
                out=kl[:], in0=kl[:], scalar1=1.0 / TAU, scalar2=None, op0=ALU.mult
            )
            nc.vector.tensor_tensor(out=kl[:], in0=kl[:], in1=rst[:], op=ALU.mult)
            nc.vector.tensor_tensor(out=kl[:], in0=kl[:], in1=lst[:], op=ALU.subtract)
            nc.vector.tensor_tensor(out=kl[:], in0=kl[:], in1=lss[:], op=ALU.add)
            # c = exp(tmax/TAU) / St
            nc.scalar.activation(out=cb[:], in_=tmx[:], func=ACTF.Exp, scale=1.0 / TAU)
            nc.vector.tensor_tensor(out=cb[:], in0=cb[:], in1=rst[:], op=ALU.mult)
            nc.vector.tensor_scalar(
                out=w[:], in0=cb[:], scalar1=float(-GAMMA),
                scalar2=float(1.0 / max(EPS, 1.0 - GAMMA)), op0=ALU.add, op1=ALU.mult,
            )
            nc.vector.tensor_scalar(
                out=w[:], in0=w[:], scalar1=0.0, scalar2=1.0, op0=ALU.max, op1=ALU.min
            )
            nc.vector.tensor_tensor(out=pk[:], in0=w[:], in1=kl[:], op=ALU.mult)
            nc.vector.tensor_scalar(
                out=pk[:], in0=pk[:], scalar1=float(TAU * TAU), scalar2=None,
                op0=ALU.mult,
            )
            nc.vector.tensor_tensor(out=pk[:], in0=pk[:], in1=keep[:], op=ALU.mult)

            nc.sync.dma_start(out=OUT[0, :, :], in_=pk[:])
            nc.sync.dma_start(out=OUT[1, :, :], in_=keep[:])
            nc.sync.dma_start(out=OUT[2, :, :], in_=mbuf[:])
            nc.sync.dma_start(out=OUT[3, :, :], in_=jf[:])
    if not nc.is_finalized():
        nc.finalize()
    return nc


# ----------------------------------------------------------------- combine
def _combine(g, outs):
    """outs: list of per-core OUT arrays [4, 128, nt] -> scalar loss."""
    loss_i = np.zeros(g.N, np.float64)
    cnt = np.zeros(g.N, np.float64)
    for c, o in enumerate(outs):
        pk, keep = np.asarray(o[0], np.float64), np.asarray(o[1], np.float64)
        for gid in range(g.n_tiles):
            lv = g.tile_live[gid]
            sm = c * g.spc + g.tile_sample[gid]
            np.add.at(loss_i, sm[lv], pk[lv, gid])
            np.add.at(cnt, sm[lv], keep[lv, gid])
    safe = np.maximum(cnt, 1.0)
    loss_i = loss_i / safe
    contrib = cnt > 0
    denom = contrib.sum()
    if denom > 0:
        return np.float32(loss_i[contrib].sum() / denom)
    return np.float32(0.0)


# ------------------------------------------------------------------- entry
_CACHE = {}


def kernel(**inputs):
    g = _plan(inputs)
    key = (g.N, g.T, g.S, g.C, tuple(g.bands),
           os.environ.get("BM_NO_GATHER"), os.environ.get("BM_NO_MAXIDX"))
    if key not in _CACHE:
        _CACHE[key] = _build(g)
    nc = _CACHE[key]
    in_maps = [
        {k: np.ascontiguousarray(v) for k, v in g.cores[c].items()}
        for c in range(N_CORES)
    ]
    res = run_bass_kernel_spmd(nc, in_maps, list(range(N_CORES)))
    outs = [res.results[c]["OUT"] for c in range(N_CORES)]
    return _combine(g, outs)


if __name__ == "__main__":
    import reference as R

    inputs = {k: np.asarray(v) for k, v in R.setup_inputs().items()}
    print("loss =", kernel(**inputs))
